# revision 11
# baseline (speedup 1.0000x reference)
"""Masked attention kernel for Trainium2, SPMD over 8 NeuronCores.

Problem: B=4, H=16, S=2048, D=64 attention with a [B,1,S,S] bool mask
(True = masked out).  The 64 (b,h) pairs are fully independent; core c
handles pairs c*8..c*8+7, which all share batch b=c//2, so each core
loads exactly one batch's mask.

Device-side math per (b,h), all in "transposed" layout (no on-device
transposes; the host pre-transposes Q/K/mask and post-transposes out):

    ST[k, q] = K @ Q^T            (fp16 matmul, f32 PSUM accumulate)
    ET[k, q] = exp(ST / 8)        (ScalarE, written as fp16)
    PT[k, q] = ET * keepT[k, q]   (VectorE; keepT = !mask as fp16 -> exact
                                   zeros for masked entries)
    OT[d, q] = V1^T @ PT          (V1 = [V | ones] -> row 64 of OT is the
                                   softmax denominator sum_k PT[k, q])

Host then returns (OT[:64] / OT[64]).T per pair.  Skipping the softmax
max-subtraction is safe: scores/8 ~ N(0,1) so exp() cannot overflow, and
masked entries are exactly zero via the keep-mask multiply.

Host-side packing puts every tensor in the exact SBUF layout so each
needs a single contiguous DMA:
  qk [npairs, 64, 2S]          rows 0:64 = [Q^T | K^T]
  v1 [npairs, 128, nchunk*65]  V chunks [128 x 65] with a ones-column
  mk [128, nchunk*S]           keep-mask chunks, concatenated along free
"""

import numpy as np
import ml_dtypes

B, H, S, D = 4, 16, 2048, 64
NCORES = 8
PAIRS_PER_CORE = (B * H) // NCORES  # 8
QTW = 512    # matmul moving-operand width (hard walrus cap)
NMASK_TT = 1  # mask multiplies per pair (DVE tensor_tensor over 1/NMASK_TT each)

F16 = np.float16
FP8 = ml_dtypes.float8_e5m2

_CACHE = {}


def build_nc(npairs=PAIRS_PER_CORE, s=S, niters=1):
    """Raw-bass build (no Tile): manual semaphores, self-loading matmuls.

    The Tile path dispatches instructions ~40x slower on the axon-tunneled
    runtime (per-instruction overhead), so this builder emits the minimal
    instruction stream with waits fused onto compute instructions.

    Pipeline per pair (per core):
      SP:  qk/v1 in-DMAs (double buffered), out-DMA (osb double buffered)
      PE:  64 scores MMs (16 chunks x 4 N=512, into 3 rotating PSUM score
           tiles of [128,1024]) then 64 PV MMs (2 q-halves x 16 chunks x 2,
           accumulating into one [65,1024] PSUM tile per half)
      ACT: 32 exps of [128,1024] PSUM->SBUF fp16
      DVE: 1 keep-mask multiply [128, 16*2048] + 2 out copies [65,1024]
    """
    import concourse.bass as bass
    import concourse.bacc as bacc
    from concourse import mybir

    nchunk = s // 128
    scw = 1024 if s >= 1024 else s   # scores psum tile width (2 banks)
    nu = s // scw                    # score units per chunk
    nmm = scw // QTW                 # MMs per score unit
    units_pp = nchunk * nu           # score units per pair
    nhalf = s // scw                 # PV halves per pair
    dt = mybir.dt

    nc = bacc.Bacc("TRN2", target_bir_lowering=False, debug=False,
                   num_devices=NCORES)

    qk_d = nc.dram_tensor("qk", [npairs, 64, 2 * s], dt.float16,
                          kind="ExternalInput")
    v1_d = nc.dram_tensor("v1", [npairs, 128, nchunk * 65], dt.float16,
                          kind="ExternalInput")
    mk_d = nc.dram_tensor("mk", [128, nchunk * s], dt.float16,
                          kind="ExternalInput")
    ot_d = nc.dram_tensor("ot", [npairs, 65, s], dt.float32,
                          kind="ExternalOutput")

    NP = npairs * niters

    with (
        nc.sbuf_tensor([128, nchunk * s], dt.float16) as mk_t,
        nc.sbuf_tensor([64, 2 * s], dt.float16) as qk0,
        nc.sbuf_tensor([64, 2 * s], dt.float16) as qk1,
        nc.sbuf_tensor([128, nchunk * 65], dt.float16) as v10,
        nc.sbuf_tensor([128, nchunk * 65], dt.float16) as v11,
        nc.sbuf_tensor([128, nchunk * s], dt.float16) as pt,
        nc.sbuf_tensor([65, s], dt.float32) as osb0,
        nc.sbuf_tensor([65, s], dt.float32) as osb1,
        nc.psum_tensor([128, scw], dt.float32) as sc0,
        nc.psum_tensor([128, scw], dt.float32) as sc1,
        nc.psum_tensor([128, scw], dt.float32) as sc2,
        nc.psum_tensor([65, scw], dt.float32) as acc,
        nc.semaphore("dma_in") as dma_in,
        nc.semaphore("sc_done") as sc_done,
        nc.semaphore("exp_done") as exp_done,
        nc.semaphore("tt_done") as tt_done,
        nc.semaphore("pv_done") as pv_done,
        nc.semaphore("cp_done") as cp_done,
        nc.semaphore("dma_out") as dma_out,
        nc.Block() as block,
    ):
        qk_b = [qk0, qk1]
        v1_b = [v10, v11]
        osb_b = [osb0, osb1]
        sc_b = [sc0, sc1, sc2]

        @block.sync
        def _(sync):
            sync.dma_start(mk_t[:], mk_d[:]).then_inc(dma_in, 16)
            nout = 0
            for p in range(NP):
                ip = p % npairs
                # self-throttle so every dma_in value others wait on is a
                # stable sync point (DMA completions may reorder otherwise)
                sync.wait_ge(dma_in, 16 + 32 * p)
                if p >= 2:
                    # qk/v1 buffer reuse: pair p-2 fully consumed
                    sync.wait_ge(sc_done, (p - 1) * units_pp)
                    sync.wait_ge(pv_done, (p - 1) * nhalf)
                sync.dma_start(qk_b[p % 2][:], qk_d[ip]).then_inc(dma_in, 16)
                sync.dma_start(v1_b[p % 2][:], v1_d[ip]).then_inc(dma_in, 16)
                if p >= 2:
                    sync.wait_ge(cp_done, (p - 1) * nhalf)
                    sync.wait_ge(dma_out, 16 * nout)
                    sync.dma_start(ot_d[(p - 2) % npairs],
                                   osb_b[p % 2][:]).then_inc(dma_out, 16)
                    nout += 1
            for p in range(max(NP - 2, 0), NP):
                sync.wait_ge(cp_done, (p + 1) * nhalf)
                sync.wait_ge(dma_out, 16 * nout)
                sync.dma_start(ot_d[p % npairs],
                               osb_b[p % 2][:]).then_inc(dma_out, 16)
                nout += 1

        @block.tensor
        def _(tensor):
            for p in range(NP):
                qk_t = qk_b[p % 2]
                v1_t = v1_b[p % 2]
                tensor.wait_ge(dma_in, 16 + 32 * (p + 1))
                for c in range(nchunk):
                    for h in range(nu):
                        u = p * units_pp + c * nu + h
                        if u >= 3:
                            tensor.wait_ge(exp_done, u - 2)
                        sc = sc_b[u % 3]
                        for j in range(nmm):
                            mm = nc.tensor.matmul(
                                sc[:, j * QTW:(j + 1) * QTW],
                                qk_t[:, s + c * 128:s + (c + 1) * 128],
                                qk_t[:, (h * nmm + j) * QTW:
                                     (h * nmm + j + 1) * QTW],
                                start=True, stop=True,
                            )
                        mm.then_inc(sc_done, 1)
                for qh in range(nhalf):
                    g = p * nhalf + qh
                    if qh == 0:
                        tensor.wait_ge(tt_done, p + 1)
                    if g >= 1:
                        tensor.wait_ge(cp_done, g)
                    for c in range(nchunk):
                        for j in range(nmm):
                            mm = nc.tensor.matmul(
                                acc[:, j * QTW:(j + 1) * QTW],
                                v1_t[:, c * 65:(c + 1) * 65],
                                pt[:, c * s + (qh * nmm + j) * QTW:
                                   c * s + (qh * nmm + j + 1) * QTW],
                                start=(c == 0), stop=(c == nchunk - 1),
                                skip_group_check=True,
                            )
                    mm.then_inc(pv_done, 1)

        @block.scalar
        def _(scalar):
            for p in range(NP):
                for c in range(nchunk):
                    for h in range(nu):
                        u = p * units_pp + c * nu + h
                        scalar.wait_ge(sc_done, u + 1)
                        if p >= 1 and c == 0 and h == 0:
                            scalar.wait_ge(pv_done, p * nhalf)
                        nc.scalar.activation(
                            pt[:, c * s + h * scw:c * s + (h + 1) * scw],
                            sc_b[u % 3][:],
                            mybir.ActivationFunctionType.Exp, scale=0.125,
                        ).then_inc(exp_done, 1)

        @block.vector
        def _(vector):
            for p in range(NP):
                vector.wait_ge(exp_done, (p + 1) * units_pp)
                if p == 0:
                    vector.wait_ge(dma_in, 48)
                nc.vector.tensor_mul(pt[:], pt[:], mk_t[:]).then_inc(tt_done, 1)
                for qh in range(nhalf):
                    g = p * nhalf + qh
                    vector.wait_ge(pv_done, g + 1)
                    if qh == 0 and p >= 2:
                        vector.wait_ge(dma_out, 16 * (p - 1))
                    nc.vector.tensor_copy(
                        osb_b[p % 2][:, qh * scw:(qh + 1) * scw], acc[:]
                    ).then_inc(cp_done, 1)

    nc.compile()
    return nc


def build_nc_tile(npairs=PAIRS_PER_CORE, s=S, niters=1):
    """Tile-scheduled build (fallback; ~40x slower dispatch on axon)."""
    import concourse.bass as bass
    import concourse.bacc as bacc
    import concourse.tile as tile
    from concourse import mybir

    nchunk = s // 128           # k chunks of 128
    nqt = s // QTW              # matmuls per chunk (scores and PV)
    dt = mybir.dt

    nc = bacc.Bacc("TRN2", target_bir_lowering=False, debug=False,
                   num_devices=NCORES)

    qk_d = nc.dram_tensor("qk", [npairs, 64, 2 * s], dt.float16,
                          kind="ExternalInput")
    v1_d = nc.dram_tensor("v1", [npairs, 128, nchunk * 65], dt.float16,
                          kind="ExternalInput")
    mk_d = nc.dram_tensor("mk", [128, nchunk * s], dt.float16,
                          kind="ExternalInput")
    ot_d = nc.dram_tensor("ot", [npairs, 65, s], dt.float32,
                          kind="ExternalOutput")

    with tile.TileContext(nc) as tc:
        with (
            tc.tile_pool(name="const", bufs=1) as const_pool,
            tc.tile_pool(name="qk", bufs=2) as qk_pool,
            tc.tile_pool(name="v", bufs=2) as v_pool,
            tc.tile_pool(name="p", bufs=1) as p_pool,
            tc.tile_pool(name="osb", bufs=2) as o_pool,
            tc.tile_pool(name="sc", bufs=1, space=bass.MemorySpace.PSUM) as sc_pool,
            tc.tile_pool(name="acc", bufs=1, space=bass.MemorySpace.PSUM) as acc_pool,
        ):
            mk_t = const_pool.tile([128, nchunk * s], dt.float16)
            nc.sync.dma_start(mk_t[:], mk_d[:])

            for p in [ip for _ in range(niters) for ip in range(npairs)]:
                qk_t = qk_pool.tile([64, 2 * s], dt.float16)
                nc.sync.dma_start(qk_t[:], qk_d[p])
                v1_t = v_pool.tile([128, nchunk * 65], dt.float16)
                nc.sync.dma_start(v1_t[:], v1_d[p])

                pt = p_pool.tile([128, nchunk * s], dt.float16)
                for c in range(nchunk):
                    sc = sc_pool.tile([128, s], dt.float32)
                    for t in range(nqt):
                        nc.tensor.matmul(
                            sc[:, t * QTW:(t + 1) * QTW],
                            qk_t[:, s + c * 128:s + (c + 1) * 128],
                            qk_t[:, t * QTW:(t + 1) * QTW],
                            start=True, stop=True,
                        )
                    nc.scalar.activation(
                        pt[:, c * s:(c + 1) * s], sc[:],
                        mybir.ActivationFunctionType.Exp, scale=0.125,
                    )
                mw = nchunk * s // NMASK_TT
                for m in range(NMASK_TT):
                    nc.vector.tensor_mul(
                        pt[:, m * mw:(m + 1) * mw],
                        pt[:, m * mw:(m + 1) * mw],
                        mk_t[:, m * mw:(m + 1) * mw],
                    )

                outp = acc_pool.tile([65, s], dt.float32)
                for c in range(nchunk):
                    for t in range(nqt):
                        nc.tensor.matmul(
                            outp[:, t * QTW:(t + 1) * QTW],
                            v1_t[:, c * 65:(c + 1) * 65],
                            pt[:, c * s + t * QTW:c * s + (t + 1) * QTW],
                            start=(c == 0), stop=(c == nchunk - 1),
                        )
                ot_sb = o_pool.tile([65, s], dt.float32)
                nc.vector.tensor_copy(ot_sb[:], outp[:])
                nc.sync.dma_start(ot_d[p], ot_sb[:])

    nc.compile()
    return nc


def _get_nc():
    key = (PAIRS_PER_CORE, S)
    if key not in _CACHE:
        _CACHE[key] = build_nc(*key)
    return _CACHE[key]


def make_core_inputs(Q, K, V, mask, core, npairs=PAIRS_PER_CORE, s=S):
    """Host-side shard prep for one core (numpy only)."""
    nchunk = s // 128
    pairs = [(f // H, f % H) for f in range(core * npairs, (core + 1) * npairs)]
    b0 = pairs[0][0]

    qk = np.empty((npairs, 64, 2 * s), dtype=F16)
    v1 = np.empty((npairs, 128, nchunk * 65), dtype=F16)
    for i, (b, h) in enumerate(pairs):
        qk[i, :, 0:s] = Q[b, h].T.astype(F16)
        qk[i, :, s:] = K[b, h].T.astype(F16)
        vc = V[b, h].astype(F16).reshape(nchunk, 128, 64).transpose(1, 0, 2)
        v1[i] = np.concatenate(
            [vc, np.ones((128, nchunk, 1), dtype=F16)], axis=2
        ).reshape(128, nchunk * 65)

    keep = (~mask[b0, 0].T).astype(F16)  # [k, q] 1.0 = keep, 0.0 = masked
    mk = np.ascontiguousarray(
        keep.reshape(nchunk, 128, s).transpose(1, 0, 2).reshape(128, nchunk * s))
    return {"qk": qk, "v1": v1, "mk": mk}


def kernel(Q, K, V, mask):
    from concourse.bass_utils import run_bass_kernel_spmd

    Q = np.asarray(Q, dtype=np.float32)
    K = np.asarray(K, dtype=np.float32)
    V = np.asarray(V, dtype=np.float32)
    mask = np.asarray(mask)

    nc = _get_nc()
    in_maps = [make_core_inputs(Q, K, V, mask, c) for c in range(NCORES)]
    res = run_bass_kernel_spmd(nc, in_maps, list(range(NCORES)))

    out = np.empty((B, H, S, D), dtype=np.float32)
    for c in range(NCORES):
        ot = res.results[c]["ot"]  # [npairs, 65, S]
        for i in range(PAIRS_PER_CORE):
            f = c * PAIRS_PER_CORE + i
            b, h = f // H, f % H
            denom = ot[i, 64:65, :]
            denom = np.where(denom == 0.0, 1.0, denom)
            out[b, h] = (ot[i, :64, :] / denom).T
    return out
